# revision 3
# baseline (speedup 1.0000x reference)
"""MoE ExpertAllocation router kernel for Trainium2 (8 NeuronCores, Bass/Tile).

Reference math (B=8, S=2048, D=4096, E=64):
  logits       = x @ W + b                         [B,S,E]
  router_probs = softmax(logits, -1)
  top_idx      = top_k(router_probs, 2).indices    [B,S,2] int32
  f_i          = one-hot-top2 counts / B
  P_i          = router_probs.sum((0,1)) / B
  aux_loss     = 0.01 * E * sum(f_i * P_i)
  capacity mask: buffer_size = (B/E)*1.25 = 0.15625 < 1, and the inclusive
  cumsum of the 0/1 routed_experts is >= 1 wherever routed_experts == 1, so
  expert_mask zeroes every routed entry:
    routed_experts (returned) == 0 and routed_probs == 0 identically.

Device work per core (tokens sharded 8 x 2048):
  - fp32 GEMM W_k [128,64] stationary, x^T [128,2048] streamed,
    logits^T accumulated in PSUM [64, 512] x 4
  - PE transpose of logits^T -> [128 tok, 64 expert] tiles
  - DVE max8/max_index -> top-2 indices (tie order matches jax top_k)
  - ACT exp(x - max) with fused row-sum; DVE reciprocal
  - P_i partial = sum_t exp[t,e] * recip[t] via tiny PE matmuls
Host: shards/transposes x, reduces P_i partials, bincounts f_i, assembles
outputs.
"""

import os
import sys

import numpy as np

for _p in ("/opt/trn_rl_repo", os.path.expanduser("~/.axon_site/_ro/trn_rl_repo")):
    if os.path.isdir(_p) and _p not in sys.path:
        sys.path.append(_p)

import concourse.bass as bass  # noqa: E402
import concourse.tile as tile  # noqa: E402
from concourse import bacc, mybir  # noqa: E402
from concourse import bass_utils  # noqa: E402
from concourse.masks import make_identity  # noqa: E402

B, S, D, E = 8, 2048, 4096, 64
N_CORES = 8
T = (B * S) // N_CORES  # tokens per core = 2048
P = 128
KT = D // P  # 32 k-tiles
NT = T // P  # 16 token tiles of 128
GF = 512  # GEMM moving free dim (fp32 max, one PSUM bank)
TB = T // GF  # 4 token blocks for the GEMM

F32 = mybir.dt.float32
U32 = mybir.dt.uint32


def _build_kernel_body(ctx, tc, xt, w, b, top_idx, p_part):
    nc = tc.nc

    const_pool = ctx.enter_context(tc.tile_pool(name="const", bufs=1))
    xpool = ctx.enter_context(tc.tile_pool(name="xtiles", bufs=3))
    gpsum = ctx.enter_context(tc.tile_pool(name="gpsum", bufs=TB, space="PSUM"))
    tpsum = ctx.enter_context(tc.tile_pool(name="tpsum", bufs=2, space="PSUM"))
    ppsum = ctx.enter_context(tc.tile_pool(name="ppsum", bufs=1, space="PSUM"))
    work = ctx.enter_context(tc.tile_pool(name="work", bufs=1))
    small = ctx.enter_context(tc.tile_pool(name="small", bufs=4))

    # Constants
    w_sb = const_pool.tile([P, KT, E], F32)
    nc.sync.dma_start(w_sb, w.rearrange("(ko p) e -> p ko e", p=P))
    b_sb = const_pool.tile([E, 1], F32)
    nc.sync.dma_start(b_sb, b[:, None])
    ident = const_pool.tile([E, E], F32)
    make_identity(nc, ident)

    # Persistent work tiles
    logitsT = work.tile([E, T], F32)
    logits3 = work.tile([P, NT, E], F32)
    ex3 = work.tile([P, NT, E], F32)
    rec = work.tile([P, NT], F32)
    idxacc = work.tile([P, NT, 2], U32)

    # ---- Router GEMM: logits^T[e, t] += W_k^T x^T_k, accumulated over k ----
    gps = [
        gpsum.tile([E, GF], F32, tag="gps", name=f"gps{i}") for i in range(TB)
    ]
    for k in range(KT):
        xt_t = xpool.tile([P, T], F32, tag="xt")
        nc.sync.dma_start(xt_t, xt[k * P : (k + 1) * P, :])
        for tb in range(TB):
            nc.tensor.matmul(
                gps[tb],
                lhsT=w_sb[:, k, :],
                rhs=xt_t[:, tb * GF : (tb + 1) * GF],
                start=(k == 0),
                stop=(k == KT - 1),
            )

    # PSUM -> SBUF with bias add (b broadcasts along tokens)
    for tb in range(TB):
        nc.vector.tensor_scalar(
            out=logitsT[:, tb * GF : (tb + 1) * GF],
            in0=gps[tb],
            scalar1=b_sb,
            scalar2=None,
            op0=mybir.AluOpType.add,
        )

    # Transpose logits^T -> [128 tok, 64 expert] tiles
    for t in range(NT):
        tp = tpsum.tile([P, E], F32, tag="tp")
        nc.tensor.transpose(tp, logitsT[:, t * P : (t + 1) * P], ident)
        nc.vector.tensor_copy(out=logits3[:, t, :], in_=tp)

    # Top-2 + softmax stats per 128-token tile
    for t in range(NT):
        lg = logits3[:, t, :]
        mx8 = small.tile([P, 8], F32, tag="mx8")
        nc.vector.max(out=mx8, in_=lg)
        ix8 = small.tile([P, 8], U32, tag="ix8")
        nc.vector.max_index(out=ix8, in_max=mx8, in_values=lg)
        nc.vector.tensor_copy(out=idxacc[:, t, :], in_=ix8[:, 0:2])
        negmx = small.tile([P, 1], F32, tag="negmx")
        nc.vector.tensor_scalar_mul(negmx, mx8[:, 0:1], -1.0)
        ssum = small.tile([P, 1], F32, tag="ssum")
        nc.scalar.activation(
            out=ex3[:, t, :],
            in_=lg,
            func=mybir.ActivationFunctionType.Exp,
            bias=negmx,
            scale=1.0,
            accum_out=ssum,
        )
        nc.vector.reciprocal(out=rec[:, t : t + 1], in_=ssum)

    # P_i partials: sum_t exp[t, e] / denom[t] via PE ones-style reduction
    pp = ppsum.tile([1, E], F32)
    for t in range(NT):
        nc.tensor.matmul(
            pp,
            lhsT=rec[:, t : t + 1],
            rhs=ex3[:, t, :],
            start=(t == 0),
            stop=(t == NT - 1),
        )
    p_sb = small.tile([1, E], F32, tag="pout")
    nc.vector.tensor_copy(out=p_sb, in_=pp)
    nc.sync.dma_start(p_part, p_sb)

    # Emit top-2 indices: SBUF [p, i, j] -> DRAM [(i p), j]
    nc.sync.dma_start(top_idx.rearrange("(i p) j -> p i j", p=P), idxacc)


_COMPILED_NC = None


def _get_compiled():
    global _COMPILED_NC
    if _COMPILED_NC is not None:
        return _COMPILED_NC
    from contextlib import ExitStack

    nc = bacc.Bacc(
        "TRN2",
        target_bir_lowering=False,
        debug=False,
        enable_asserts=False,
        num_devices=N_CORES,
    )
    xt = nc.dram_tensor("xt", [D, T], F32, kind="ExternalInput").ap()
    w = nc.dram_tensor("w", [D, E], F32, kind="ExternalInput").ap()
    b = nc.dram_tensor("b", [E], F32, kind="ExternalInput").ap()
    top_idx = nc.dram_tensor("top_idx", [T, 2], U32, kind="ExternalOutput").ap()
    p_part = nc.dram_tensor("p_part", [1, E], F32, kind="ExternalOutput").ap()

    with tile.TileContext(nc) as tc:
        with ExitStack() as ctx:
            _build_kernel_body(ctx, tc, xt, w, b, top_idx, p_part)
    nc.compile()
    _COMPILED_NC = nc
    return nc


def _run_device(x, W, b, trace=False):
    nc = _get_compiled()
    xf = np.ascontiguousarray(np.asarray(x, dtype=np.float32)).reshape(B * S, D)
    Wf = np.ascontiguousarray(np.asarray(W, dtype=np.float32))
    bf = np.ascontiguousarray(np.asarray(b, dtype=np.float32))
    in_maps = []
    for c in range(N_CORES):
        shard = np.ascontiguousarray(xf[c * T : (c + 1) * T, :].T)
        in_maps.append({"xt": shard, "w": Wf, "b": bf})
    res = bass_utils.run_bass_kernel_spmd(
        nc, in_maps, core_ids=list(range(N_CORES)), trace=trace
    )
    return res


def kernel(x, W, b):
    res = _run_device(x, W, b, trace=False)
    return _assemble(res.results)


def _assemble(results):
    top_u = np.concatenate(
        [results[c]["top_idx"] for c in range(N_CORES)], axis=0
    )  # [B*S, 2] uint32
    top_idx = top_u.astype(np.int32).reshape(B, S, 2)

    p_sum = np.zeros(E, dtype=np.float64)
    for c in range(N_CORES):
        p_sum += results[c]["p_part"][0].astype(np.float64)
    P_i = p_sum / float(B)
    f_i = np.bincount(top_u.ravel().astype(np.int64), minlength=E).astype(
        np.float64
    ) / float(B)
    aux_loss = np.float32(0.01 * E * np.sum(f_i * P_i))

    routed_experts = np.zeros((B, S, E), dtype=np.float32)
    routed_probs = np.zeros((B, S, E), dtype=np.float32)
    return routed_experts, routed_probs, top_idx, aux_loss


# revision 6
# speedup vs baseline: 1.0171x; 1.0171x over previous
"""MoE ExpertAllocation router kernel for Trainium2 (8 NeuronCores, Bass/Tile).

Reference math (B=8, S=2048, D=4096, E=64):
  logits       = x @ W + b                         [B,S,E]
  router_probs = softmax(logits, -1)
  top_idx      = top_k(router_probs, 2).indices    [B,S,2] int32
  f_i          = one-hot-top2 counts / B
  P_i          = router_probs.sum((0,1)) / B
  aux_loss     = 0.01 * E * sum(f_i * P_i)
  capacity mask: buffer_size = (B/E)*1.25 = 0.15625 < 1, and the inclusive
  cumsum of the 0/1 routed_experts is >= 1 wherever routed_experts == 1, so
  expert_mask zeroes every routed entry:
    routed_experts (returned) == 0 and routed_probs == 0 identically.

Device work per core (tokens sharded 8 x 2048):
  - fp32 GEMM W_k [128,64] stationary, x^T [128,2048] streamed,
    logits^T accumulated in PSUM [64, 512] x 4
  - PE transpose of logits^T -> [128 tok, 64 expert] tiles
  - DVE max8/max_index -> top-2 indices (tie order matches jax top_k)
  - ACT exp(x - max) with fused row-sum; DVE reciprocal
  - P_i partial = sum_t exp[t,e] * recip[t] via tiny PE matmuls
Host: shards/transposes x, reduces P_i partials, bincounts f_i, assembles
outputs.
"""

import os
import sys

import numpy as np

for _p in ("/opt/trn_rl_repo", os.path.expanduser("~/.axon_site/_ro/trn_rl_repo")):
    if os.path.isdir(_p) and _p not in sys.path:
        sys.path.append(_p)

import concourse.bass as bass  # noqa: E402
import concourse.tile as tile  # noqa: E402
from concourse import bacc, mybir  # noqa: E402
from concourse import bass_utils  # noqa: E402
from concourse.masks import make_identity  # noqa: E402

B, S, D, E = 8, 2048, 4096, 64
N_CORES = 8
T = (B * S) // N_CORES  # tokens per core = 2048
P = 128
KT = D // P  # 32 k-tiles
NT = T // P  # 16 token tiles of 128
GF = 512  # GEMM moving free dim (fp32 max, one PSUM bank)
TB = T // GF  # 4 token blocks for the GEMM

F32 = mybir.dt.float32
U32 = mybir.dt.uint32


def _build_kernel_body(ctx, tc, xt, w, b, top_idx, p_part, col_tile=True):
    nc = tc.nc

    const_pool = ctx.enter_context(tc.tile_pool(name="const", bufs=1))
    wpool = ctx.enter_context(tc.tile_pool(name="wk", bufs=KT))
    xpool = ctx.enter_context(tc.tile_pool(name="xtiles", bufs=4))
    gpsum = ctx.enter_context(tc.tile_pool(name="gpsum", bufs=TB, space="PSUM"))
    tpsum = ctx.enter_context(tc.tile_pool(name="tpsum", bufs=2, space="PSUM"))
    ppsum = ctx.enter_context(tc.tile_pool(name="ppsum", bufs=1, space="PSUM"))
    work = ctx.enter_context(tc.tile_pool(name="work", bufs=1))
    small = ctx.enter_context(tc.tile_pool(name="small", bufs=4))

    # Constants
    b_sb = const_pool.tile([E, 1], F32)
    nc.sync.dma_start(b_sb, b[:, None])
    ident = const_pool.tile([E, E], F32)
    make_identity(nc, ident)

    # Per-k W tiles (contiguous 32KB loads; precise deps so k=0 starts early)
    wk = []
    for k in range(KT):
        w_t = wpool.tile([P, E], F32, tag="wk", name=f"wk{k}")
        nc.sync.dma_start(w_t, w[k * P : (k + 1) * P, :])
        wk.append(w_t)

    # Persistent work tiles
    logitsT = work.tile([E, T], F32)
    logits3 = work.tile([P, NT, E], F32)
    ex3 = work.tile([P, NT, E], F32)
    rec = work.tile([P, NT], F32)
    idxacc = work.tile([P, NT, 2], U32)

    # ---- Router GEMM: logits^T[e, t] += W_k^T x^T_k, accumulated over k ----
    if col_tile:
        # Two k-chains run concurrently on PE column halves h0/h1; the two
        # partial logit^T halves are summed during PSUM evacuation.
        gps = [
            gpsum.tile([P, GF], F32, tag="gps", name=f"gps{i}") for i in range(TB)
        ]
        KP = KT // 2
        for j in range(KP):
            k0, k1 = 2 * j, 2 * j + 1
            xt_e = xpool.tile([P, T], F32, tag="xt", name=f"xte{j}")
            nc.sync.dma_start(xt_e, xt[k0 * P : (k0 + 1) * P, :])
            xt_o = xpool.tile([P, T], F32, tag="xt", name=f"xto{j}")
            nc.sync.dma_start(xt_o, xt[k1 * P : (k1 + 1) * P, :])
            for tb in range(TB):
                nc.tensor.matmul(
                    gps[tb][0:E, :],
                    lhsT=wk[k0],
                    rhs=xt_e[:, tb * GF : (tb + 1) * GF],
                    start=(j == 0),
                    stop=(j == KP - 1),
                    tile_position=(0, 0),
                    skip_group_check=True,
                )
                nc.tensor.matmul(
                    gps[tb][E : 2 * E, :],
                    lhsT=wk[k1],
                    rhs=xt_o[:, tb * GF : (tb + 1) * GF],
                    start=(j == 0),
                    stop=(j == KP - 1),
                    tile_position=(0, E),
                    skip_group_check=True,
                )
        # logitsT = (h0 + b) + h1 (DVE may read only one PSUM operand per op)
        for tb in range(TB):
            seg = logitsT[:, tb * GF : (tb + 1) * GF]
            nc.vector.tensor_scalar(
                out=seg,
                in0=gps[tb][0:E, :],
                scalar1=b_sb,
                scalar2=None,
                op0=mybir.AluOpType.add,
            )
            nc.vector.tensor_add(out=seg, in0=seg, in1=gps[tb][E : 2 * E, :])
    else:
        gps = [
            gpsum.tile([E, GF], F32, tag="gps", name=f"gps{i}") for i in range(TB)
        ]
        for k in range(KT):
            xt_t = xpool.tile([P, T], F32, tag="xt")
            nc.sync.dma_start(xt_t, xt[k * P : (k + 1) * P, :])
            for tb in range(TB):
                nc.tensor.matmul(
                    gps[tb],
                    lhsT=wk[k],
                    rhs=xt_t[:, tb * GF : (tb + 1) * GF],
                    start=(k == 0),
                    stop=(k == KT - 1),
                )
        # PSUM -> SBUF with bias add (b broadcasts along tokens)
        for tb in range(TB):
            nc.vector.tensor_scalar(
                out=logitsT[:, tb * GF : (tb + 1) * GF],
                in0=gps[tb],
                scalar1=b_sb,
                scalar2=None,
                op0=mybir.AluOpType.add,
            )

    # Transpose logits^T -> [128 tok, 64 expert] tiles
    for t in range(NT):
        tp = tpsum.tile([P, E], F32, tag="tp")
        nc.tensor.transpose(tp, logitsT[:, t * P : (t + 1) * P], ident)
        nc.vector.tensor_copy(out=logits3[:, t, :], in_=tp)

    # Top-2 + softmax stats per 128-token tile
    for t in range(NT):
        lg = logits3[:, t, :]
        mx8 = small.tile([P, 8], F32, tag="mx8")
        nc.vector.max(out=mx8, in_=lg)
        ix8 = small.tile([P, 8], U32, tag="ix8")
        nc.vector.max_index(out=ix8, in_max=mx8, in_values=lg)
        nc.vector.tensor_copy(out=idxacc[:, t, :], in_=ix8[:, 0:2])
        negmx = small.tile([P, 1], F32, tag="negmx")
        nc.vector.tensor_scalar_mul(negmx, mx8[:, 0:1], -1.0)
        ssum = small.tile([P, 1], F32, tag="ssum")
        nc.scalar.activation(
            out=ex3[:, t, :],
            in_=lg,
            func=mybir.ActivationFunctionType.Exp,
            bias=negmx,
            scale=1.0,
            accum_out=ssum,
        )
        nc.vector.reciprocal(out=rec[:, t : t + 1], in_=ssum)

    # P_i partials: sum_t exp[t, e] / denom[t] via PE ones-style reduction
    pp = ppsum.tile([1, E], F32)
    for t in range(NT):
        nc.tensor.matmul(
            pp,
            lhsT=rec[:, t : t + 1],
            rhs=ex3[:, t, :],
            start=(t == 0),
            stop=(t == NT - 1),
        )
    p_sb = small.tile([1, E], F32, tag="pout")
    nc.vector.tensor_copy(out=p_sb, in_=pp)
    nc.sync.dma_start(p_part, p_sb)

    # Emit top-2 indices: SBUF [p, i, j] -> DRAM [(i p), j]
    nc.sync.dma_start(top_idx.rearrange("(i p) j -> p i j", p=P), idxacc)


_COMPILED_NC = None


def _get_compiled():
    global _COMPILED_NC
    if _COMPILED_NC is not None:
        return _COMPILED_NC
    from contextlib import ExitStack

    nc = bacc.Bacc(
        "TRN2",
        target_bir_lowering=False,
        debug=False,
        enable_asserts=False,
        num_devices=N_CORES,
    )
    xt = nc.dram_tensor("xt", [D, T], F32, kind="ExternalInput").ap()
    w = nc.dram_tensor("w", [D, E], F32, kind="ExternalInput").ap()
    b = nc.dram_tensor("b", [E], F32, kind="ExternalInput").ap()
    top_idx = nc.dram_tensor("top_idx", [T, 2], U32, kind="ExternalOutput").ap()
    p_part = nc.dram_tensor("p_part", [1, E], F32, kind="ExternalOutput").ap()

    with tile.TileContext(nc) as tc:
        with ExitStack() as ctx:
            _build_kernel_body(ctx, tc, xt, w, b, top_idx, p_part)
    nc.compile()
    _COMPILED_NC = nc
    return nc


def _run_device(x, W, b, trace=False):
    nc = _get_compiled()
    xf = np.ascontiguousarray(np.asarray(x, dtype=np.float32)).reshape(B * S, D)
    Wf = np.ascontiguousarray(np.asarray(W, dtype=np.float32))
    bf = np.ascontiguousarray(np.asarray(b, dtype=np.float32))
    in_maps = []
    for c in range(N_CORES):
        shard = np.ascontiguousarray(xf[c * T : (c + 1) * T, :].T)
        in_maps.append({"xt": shard, "w": Wf, "b": bf})
    res = bass_utils.run_bass_kernel_spmd(
        nc, in_maps, core_ids=list(range(N_CORES)), trace=trace
    )
    return res


def kernel(x, W, b):
    res = _run_device(x, W, b, trace=False)
    return _assemble(res.results)


def _assemble(results):
    top_u = np.concatenate(
        [results[c]["top_idx"] for c in range(N_CORES)], axis=0
    )  # [B*S, 2] uint32
    top_idx = top_u.astype(np.int32).reshape(B, S, 2)

    p_sum = np.zeros(E, dtype=np.float64)
    for c in range(N_CORES):
        p_sum += results[c]["p_part"][0].astype(np.float64)
    P_i = p_sum / float(B)
    f_i = np.bincount(top_u.ravel().astype(np.int64), minlength=E).astype(
        np.float64
    ) / float(B)
    aux_loss = np.float32(0.01 * E * np.sum(f_i * P_i))

    routed_experts = np.zeros((B, S, E), dtype=np.float32)
    routed_probs = np.zeros((B, S, E), dtype=np.float32)
    return routed_experts, routed_probs, top_idx, aux_loss


# revision 8
# speedup vs baseline: 1.0884x; 1.0702x over previous
"""MoE ExpertAllocation router kernel for Trainium2 (8 NeuronCores, Bass/Tile).

Reference math (B=8, S=2048, D=4096, E=64):
  logits       = x @ W + b                         [B,S,E]
  router_probs = softmax(logits, -1)
  top_idx      = top_k(router_probs, 2).indices    [B,S,2] int32
  f_i          = one-hot-top2 counts / B
  P_i          = router_probs.sum((0,1)) / B
  aux_loss     = 0.01 * E * sum(f_i * P_i)
  capacity mask: buffer_size = (B/E)*1.25 = 0.15625 < 1, and the inclusive
  cumsum of the 0/1 routed_experts is >= 1 wherever routed_experts == 1, so
  expert_mask zeroes every routed entry:
    routed_experts (returned) == 0 and routed_probs == 0 identically.

Device work per core (tokens sharded 8 x 2048):
  - fp32 GEMM W_k [128,64] stationary, x^T [128,2048] streamed,
    logits^T accumulated in PSUM [64, 512] x 4
  - PE transpose of logits^T -> [128 tok, 64 expert] tiles
  - DVE max8/max_index -> top-2 indices (tie order matches jax top_k)
  - ACT exp(x - max) with fused row-sum; DVE reciprocal
  - P_i partial = sum_t exp[t,e] * recip[t] via tiny PE matmuls
Host: shards/transposes x, reduces P_i partials, bincounts f_i, assembles
outputs.
"""

import os
import sys

import numpy as np

for _p in ("/opt/trn_rl_repo", os.path.expanduser("~/.axon_site/_ro/trn_rl_repo")):
    if os.path.isdir(_p) and _p not in sys.path:
        sys.path.append(_p)

import concourse.bass as bass  # noqa: E402
import concourse.tile as tile  # noqa: E402
from concourse import bacc, mybir  # noqa: E402
from concourse import bass_utils  # noqa: E402
from concourse.masks import make_identity  # noqa: E402

B, S, D, E = 8, 2048, 4096, 64
N_CORES = 8
T = (B * S) // N_CORES  # tokens per core = 2048
P = 128
KT = D // P  # 32 k-tiles
NT = T // P  # 16 token tiles of 128
GF = 512  # GEMM moving free dim (fp32 max, one PSUM bank)
TB = T // GF  # 4 token blocks for the GEMM

F32 = mybir.dt.float32
U32 = mybir.dt.uint32


def _build_kernel_body(ctx, tc, xt, w, b, top_idx, p_part, col_tile=True):
    nc = tc.nc

    const_pool = ctx.enter_context(tc.tile_pool(name="const", bufs=1))
    wpool = ctx.enter_context(tc.tile_pool(name="wk", bufs=KT))
    xpool = ctx.enter_context(tc.tile_pool(name="xtiles", bufs=6))
    gpsum = ctx.enter_context(tc.tile_pool(name="gpsum", bufs=TB, space="PSUM"))
    tpsum = ctx.enter_context(tc.tile_pool(name="tpsum", bufs=2, space="PSUM"))
    ppsum = ctx.enter_context(tc.tile_pool(name="ppsum", bufs=1, space="PSUM"))
    work = ctx.enter_context(tc.tile_pool(name="work", bufs=1))
    small = ctx.enter_context(tc.tile_pool(name="small", bufs=4))

    # Constants and W tiles ride the GpSimd SWDGE queue so the Sync HWDGE
    # queue starts streaming x^T immediately (32 serial wk triggers on Sync
    # were delaying the first GEMM matmul by ~25us).
    b_sb = const_pool.tile([E, 1], F32)
    nc.gpsimd.dma_start(b_sb, b[:, None])
    ident = const_pool.tile([E, E], F32)
    make_identity(nc, ident)

    # Per-k W tiles (contiguous 32KB loads; precise deps so k=0 starts early)
    wk = []
    for k in range(KT):
        w_t = wpool.tile([P, E], F32, tag="wk", name=f"wk{k}")
        nc.gpsimd.dma_start(w_t, w[k * P : (k + 1) * P, :])
        wk.append(w_t)

    # Persistent work tiles
    logitsT = work.tile([E, T], F32)
    logits3 = work.tile([P, NT, E], F32)
    ex3 = work.tile([P, NT, E], F32)
    rec = work.tile([P, NT], F32)
    idxacc = work.tile([P, NT, 2], U32)

    # ---- Router GEMM: logits^T[e, t] += W_k^T x^T_k, accumulated over k ----
    if col_tile:
        # Two k-chains run concurrently on PE column halves h0/h1; the two
        # partial logit^T halves are summed during PSUM evacuation.
        gps = [
            gpsum.tile([P, GF], F32, tag="gps", name=f"gps{i}") for i in range(TB)
        ]
        KP = KT // 2
        for j in range(KP):
            k0, k1 = 2 * j, 2 * j + 1
            xt_e = xpool.tile([P, T], F32, tag="xt", name=f"xte{j}")
            nc.sync.dma_start(xt_e, xt[k0 * P : (k0 + 1) * P, :])
            xt_o = xpool.tile([P, T], F32, tag="xt", name=f"xto{j}")
            nc.sync.dma_start(xt_o, xt[k1 * P : (k1 + 1) * P, :])
            for tb in range(TB):
                nc.tensor.matmul(
                    gps[tb][0:E, :],
                    lhsT=wk[k0],
                    rhs=xt_e[:, tb * GF : (tb + 1) * GF],
                    start=(j == 0),
                    stop=(j == KP - 1),
                    tile_position=(0, 0),
                    skip_group_check=True,
                )
                nc.tensor.matmul(
                    gps[tb][E : 2 * E, :],
                    lhsT=wk[k1],
                    rhs=xt_o[:, tb * GF : (tb + 1) * GF],
                    start=(j == 0),
                    stop=(j == KP - 1),
                    tile_position=(0, E),
                    skip_group_check=True,
                )
        # logitsT = (h0 + b) + h1 (DVE may read only one PSUM operand per op)
        for tb in range(TB):
            seg = logitsT[:, tb * GF : (tb + 1) * GF]
            nc.vector.tensor_scalar(
                out=seg,
                in0=gps[tb][0:E, :],
                scalar1=b_sb,
                scalar2=None,
                op0=mybir.AluOpType.add,
            )
            nc.vector.tensor_add(out=seg, in0=seg, in1=gps[tb][E : 2 * E, :])
    else:
        gps = [
            gpsum.tile([E, GF], F32, tag="gps", name=f"gps{i}") for i in range(TB)
        ]
        for k in range(KT):
            xt_t = xpool.tile([P, T], F32, tag="xt")
            nc.sync.dma_start(xt_t, xt[k * P : (k + 1) * P, :])
            for tb in range(TB):
                nc.tensor.matmul(
                    gps[tb],
                    lhsT=wk[k],
                    rhs=xt_t[:, tb * GF : (tb + 1) * GF],
                    start=(k == 0),
                    stop=(k == KT - 1),
                )
        # PSUM -> SBUF with bias add (b broadcasts along tokens)
        for tb in range(TB):
            nc.vector.tensor_scalar(
                out=logitsT[:, tb * GF : (tb + 1) * GF],
                in0=gps[tb],
                scalar1=b_sb,
                scalar2=None,
                op0=mybir.AluOpType.add,
            )

    # Transpose logits^T -> [128 tok, 64 expert] tiles.  The PSUM->SBUF
    # copy rides ScalarE and small copies ride GpSimd, keeping DVE free for
    # max8/find_index8/reciprocal (the tail-phase critical path).
    for t in range(NT):
        tp = tpsum.tile([P, E], F32, tag="tp")
        nc.tensor.transpose(tp, logitsT[:, t * P : (t + 1) * P], ident)
        nc.scalar.copy(out=logits3[:, t, :], in_=tp)

    # Top-2 + softmax stats per 128-token tile
    for t in range(NT):
        lg = logits3[:, t, :]
        mx8 = small.tile([P, 8], F32, tag="mx8")
        nc.vector.max(out=mx8, in_=lg)
        ix8 = small.tile([P, 8], U32, tag="ix8")
        nc.vector.max_index(out=ix8, in_max=mx8, in_values=lg)
        nc.gpsimd.tensor_copy(out=idxacc[:, t, :], in_=ix8[:, 0:2])
        negmx = small.tile([P, 1], F32, tag="negmx")
        nc.gpsimd.tensor_scalar_mul(negmx, mx8[:, 0:1], -1.0)
        ssum = small.tile([P, 1], F32, tag="ssum")
        nc.scalar.activation(
            out=ex3[:, t, :],
            in_=lg,
            func=mybir.ActivationFunctionType.Exp,
            bias=negmx,
            scale=1.0,
            accum_out=ssum,
        )
        nc.vector.reciprocal(out=rec[:, t : t + 1], in_=ssum)

    # P_i partials: sum_t exp[t, e] / denom[t] via PE ones-style reduction
    pp = ppsum.tile([1, E], F32)
    for t in range(NT):
        nc.tensor.matmul(
            pp,
            lhsT=rec[:, t : t + 1],
            rhs=ex3[:, t, :],
            start=(t == 0),
            stop=(t == NT - 1),
        )
    p_sb = small.tile([1, E], F32, tag="pout")
    nc.vector.tensor_copy(out=p_sb, in_=pp)
    nc.sync.dma_start(p_part, p_sb)

    # Emit top-2 indices: SBUF [p, i, j] -> DRAM [(i p), j]
    nc.sync.dma_start(top_idx.rearrange("(i p) j -> p i j", p=P), idxacc)


_COMPILED_NC = None


def _get_compiled():
    global _COMPILED_NC
    if _COMPILED_NC is not None:
        return _COMPILED_NC
    from contextlib import ExitStack

    nc = bacc.Bacc(
        "TRN2",
        target_bir_lowering=False,
        debug=False,
        enable_asserts=False,
        num_devices=N_CORES,
    )
    xt = nc.dram_tensor("xt", [D, T], F32, kind="ExternalInput").ap()
    w = nc.dram_tensor("w", [D, E], F32, kind="ExternalInput").ap()
    b = nc.dram_tensor("b", [E], F32, kind="ExternalInput").ap()
    top_idx = nc.dram_tensor("top_idx", [T, 2], U32, kind="ExternalOutput").ap()
    p_part = nc.dram_tensor("p_part", [1, E], F32, kind="ExternalOutput").ap()

    with tile.TileContext(nc) as tc:
        with ExitStack() as ctx:
            _build_kernel_body(ctx, tc, xt, w, b, top_idx, p_part)
    nc.compile()
    _COMPILED_NC = nc
    return nc


def _run_device(x, W, b, trace=False):
    nc = _get_compiled()
    xf = np.ascontiguousarray(np.asarray(x, dtype=np.float32)).reshape(B * S, D)
    Wf = np.ascontiguousarray(np.asarray(W, dtype=np.float32))
    bf = np.ascontiguousarray(np.asarray(b, dtype=np.float32))
    in_maps = []
    for c in range(N_CORES):
        shard = np.ascontiguousarray(xf[c * T : (c + 1) * T, :].T)
        in_maps.append({"xt": shard, "w": Wf, "b": bf})
    res = bass_utils.run_bass_kernel_spmd(
        nc, in_maps, core_ids=list(range(N_CORES)), trace=trace
    )
    return res


def kernel(x, W, b):
    res = _run_device(x, W, b, trace=False)
    return _assemble(res.results)


def _assemble(results):
    top_u = np.concatenate(
        [results[c]["top_idx"] for c in range(N_CORES)], axis=0
    )  # [B*S, 2] uint32
    top_idx = top_u.astype(np.int32).reshape(B, S, 2)

    p_sum = np.zeros(E, dtype=np.float64)
    for c in range(N_CORES):
        p_sum += results[c]["p_part"][0].astype(np.float64)
    P_i = p_sum / float(B)
    f_i = np.bincount(top_u.ravel().astype(np.int64), minlength=E).astype(
        np.float64
    ) / float(B)
    aux_loss = np.float32(0.01 * E * np.sum(f_i * P_i))

    routed_experts = np.zeros((B, S, E), dtype=np.float32)
    routed_probs = np.zeros((B, S, E), dtype=np.float32)
    return routed_experts, routed_probs, top_idx, aux_loss


# revision 10
# speedup vs baseline: 1.1458x; 1.0527x over previous
"""MoE ExpertAllocation router kernel for Trainium2 (8 NeuronCores, Bass/Tile).

Reference math (B=8, S=2048, D=4096, E=64):
  logits       = x @ W + b                         [B,S,E]
  router_probs = softmax(logits, -1)
  top_idx      = top_k(router_probs, 2).indices    [B,S,2] int32
  f_i          = one-hot-top2 counts / B
  P_i          = router_probs.sum((0,1)) / B
  aux_loss     = 0.01 * E * sum(f_i * P_i)
  capacity mask: buffer_size = (B/E)*1.25 = 0.15625 < 1, and the inclusive
  cumsum of the 0/1 routed_experts is >= 1 wherever routed_experts == 1, so
  expert_mask zeroes every routed entry:
    routed_experts (returned) == 0 and routed_probs == 0 identically.

Device work per core (tokens sharded 8 x 2048):
  - fp32 GEMM W_k [128,64] stationary, x^T [128,2048] streamed,
    logits^T accumulated in PSUM [64, 512] x 4
  - PE transpose of logits^T -> [128 tok, 64 expert] tiles
  - DVE max8/max_index -> top-2 indices (tie order matches jax top_k)
  - ACT exp(x - max) with fused row-sum; DVE reciprocal
  - P_i partial = sum_t exp[t,e] * recip[t] via tiny PE matmuls
Host: shards/transposes x, reduces P_i partials, bincounts f_i, assembles
outputs.
"""

import os
import sys

import numpy as np

for _p in ("/opt/trn_rl_repo", os.path.expanduser("~/.axon_site/_ro/trn_rl_repo")):
    if os.path.isdir(_p) and _p not in sys.path:
        sys.path.append(_p)

import concourse.bass as bass  # noqa: E402
import concourse.tile as tile  # noqa: E402
from concourse import bacc, mybir  # noqa: E402
from concourse import bass_utils  # noqa: E402
from concourse.masks import make_identity  # noqa: E402

B, S, D, E = 8, 2048, 4096, 64
N_CORES = 8
T = (B * S) // N_CORES  # tokens per core = 2048
P = 128
KT = D // P  # 32 k-tiles
NT = T // P  # 16 token tiles of 128
GF = 512  # GEMM moving free dim (fp32 max, one PSUM bank)
TB = T // GF  # 4 token blocks for the GEMM

F32 = mybir.dt.float32
U32 = mybir.dt.uint32


def _build_kernel_body(ctx, tc, xt, w, b, top_idx, p_part, col_tile=True):
    nc = tc.nc

    const_pool = ctx.enter_context(tc.tile_pool(name="const", bufs=1))
    wpool = ctx.enter_context(tc.tile_pool(name="wk", bufs=4))
    xpool = ctx.enter_context(tc.tile_pool(name="xtiles", bufs=6))
    gpsum = ctx.enter_context(tc.tile_pool(name="gpsum", bufs=TB, space="PSUM"))
    tpsum = ctx.enter_context(tc.tile_pool(name="tpsum", bufs=2, space="PSUM"))
    ppsum = ctx.enter_context(tc.tile_pool(name="ppsum", bufs=1, space="PSUM"))
    work = ctx.enter_context(tc.tile_pool(name="work", bufs=1))
    small = ctx.enter_context(tc.tile_pool(name="small", bufs=4))

    # Constants ride the GpSimd SWDGE queue and W rides the ScalarE HWDGE
    # ring (qActDynamicHW), keeping the Sync ring free to stream x^T from
    # the first instruction (wk triggers on Sync delayed the GEMM ~25us;
    # wk on the slow GpSimd SWDGE queue weight-starved it instead).
    b_sb = const_pool.tile([E, 1], F32)
    nc.gpsimd.dma_start(b_sb, b[:, None])
    ident = const_pool.tile([E, E], F32)
    make_identity(nc, ident)

    # W in 4 k-groups of 8: few triggers, early availability, per-group deps
    WG = 8
    wgroups = []
    for g in range(KT // WG):
        w_t = wpool.tile([P, WG, E], F32, tag="wg", name=f"wg{g}")
        nc.scalar.dma_start(
            w_t,
            w[g * WG * P : (g + 1) * WG * P, :].rearrange(
                "(ko p) e -> p ko e", p=P
            ),
        )
        wgroups.append(w_t)
    wk = [wgroups[k // WG][:, k % WG, :] for k in range(KT)]

    # Persistent work tiles
    logitsT = work.tile([E, T], F32)
    logits3 = work.tile([P, NT, E], F32)
    ex3 = work.tile([P, NT, E], F32)
    rec = work.tile([P, NT], F32)
    idxacc = work.tile([P, NT, 2], U32)

    # ---- Router GEMM: logits^T[e, t] += W_k^T x^T_k, accumulated over k ----
    if col_tile:
        # Two k-chains run concurrently on PE column halves h0/h1; the two
        # partial logit^T halves are summed during PSUM evacuation.
        gps = [
            gpsum.tile([P, GF], F32, tag="gps", name=f"gps{i}") for i in range(TB)
        ]
        KP = KT // 2
        for j in range(KP):
            k0, k1 = 2 * j, 2 * j + 1
            xt_e = xpool.tile([P, T], F32, tag="xt", name=f"xte{j}")
            nc.sync.dma_start(xt_e, xt[k0 * P : (k0 + 1) * P, :])
            xt_o = xpool.tile([P, T], F32, tag="xt", name=f"xto{j}")
            nc.sync.dma_start(xt_o, xt[k1 * P : (k1 + 1) * P, :])
            for tb in range(TB):
                nc.tensor.matmul(
                    gps[tb][0:E, :],
                    lhsT=wk[k0],
                    rhs=xt_e[:, tb * GF : (tb + 1) * GF],
                    start=(j == 0),
                    stop=(j == KP - 1),
                    tile_position=(0, 0),
                    skip_group_check=True,
                )
                nc.tensor.matmul(
                    gps[tb][E : 2 * E, :],
                    lhsT=wk[k1],
                    rhs=xt_o[:, tb * GF : (tb + 1) * GF],
                    start=(j == 0),
                    stop=(j == KP - 1),
                    tile_position=(0, E),
                    skip_group_check=True,
                )
        # logitsT = (h0 + b) + h1 (DVE may read only one PSUM operand per op)
        for tb in range(TB):
            seg = logitsT[:, tb * GF : (tb + 1) * GF]
            nc.vector.tensor_scalar(
                out=seg,
                in0=gps[tb][0:E, :],
                scalar1=b_sb,
                scalar2=None,
                op0=mybir.AluOpType.add,
            )
            nc.vector.tensor_add(out=seg, in0=seg, in1=gps[tb][E : 2 * E, :])
    else:
        gps = [
            gpsum.tile([E, GF], F32, tag="gps", name=f"gps{i}") for i in range(TB)
        ]
        for k in range(KT):
            xt_t = xpool.tile([P, T], F32, tag="xt")
            nc.sync.dma_start(xt_t, xt[k * P : (k + 1) * P, :])
            for tb in range(TB):
                nc.tensor.matmul(
                    gps[tb],
                    lhsT=wk[k],
                    rhs=xt_t[:, tb * GF : (tb + 1) * GF],
                    start=(k == 0),
                    stop=(k == KT - 1),
                )
        # PSUM -> SBUF with bias add (b broadcasts along tokens)
        for tb in range(TB):
            nc.vector.tensor_scalar(
                out=logitsT[:, tb * GF : (tb + 1) * GF],
                in0=gps[tb],
                scalar1=b_sb,
                scalar2=None,
                op0=mybir.AluOpType.add,
            )

    # Transpose logits^T -> [128 tok, 64 expert] tiles.  The PSUM->SBUF
    # copy rides ScalarE and small copies ride GpSimd, keeping DVE free for
    # max8/find_index8/reciprocal (the tail-phase critical path).
    for t in range(NT):
        tp = tpsum.tile([P, E], F32, tag="tp")
        nc.tensor.transpose(tp, logitsT[:, t * P : (t + 1) * P], ident)
        nc.scalar.copy(out=logits3[:, t, :], in_=tp)

    # Top-2 + softmax stats per 128-token tile
    for t in range(NT):
        lg = logits3[:, t, :]
        mx8 = small.tile([P, 8], F32, tag="mx8")
        nc.vector.max(out=mx8, in_=lg)
        ix8 = small.tile([P, 8], U32, tag="ix8")
        nc.vector.max_index(out=ix8, in_max=mx8, in_values=lg)
        nc.gpsimd.tensor_copy(out=idxacc[:, t, :], in_=ix8[:, 0:2])
        negmx = small.tile([P, 1], F32, tag="negmx")
        nc.gpsimd.tensor_scalar_mul(negmx, mx8[:, 0:1], -1.0)
        ssum = small.tile([P, 1], F32, tag="ssum")
        nc.scalar.activation(
            out=ex3[:, t, :],
            in_=lg,
            func=mybir.ActivationFunctionType.Exp,
            bias=negmx,
            scale=1.0,
            accum_out=ssum,
        )
        nc.vector.reciprocal(out=rec[:, t : t + 1], in_=ssum)

    # P_i partials: sum_t exp[t, e] / denom[t] via PE ones-style reduction
    pp = ppsum.tile([1, E], F32)
    for t in range(NT):
        nc.tensor.matmul(
            pp,
            lhsT=rec[:, t : t + 1],
            rhs=ex3[:, t, :],
            start=(t == 0),
            stop=(t == NT - 1),
        )
    p_sb = small.tile([1, E], F32, tag="pout")
    nc.vector.tensor_copy(out=p_sb, in_=pp)
    nc.sync.dma_start(p_part, p_sb)

    # Emit top-2 indices: SBUF [p, i, j] -> DRAM [(i p), j]
    nc.sync.dma_start(top_idx.rearrange("(i p) j -> p i j", p=P), idxacc)


_COMPILED_NC = None


def _get_compiled():
    global _COMPILED_NC
    if _COMPILED_NC is not None:
        return _COMPILED_NC
    from contextlib import ExitStack

    nc = bacc.Bacc(
        "TRN2",
        target_bir_lowering=False,
        debug=False,
        enable_asserts=False,
        num_devices=N_CORES,
    )
    xt = nc.dram_tensor("xt", [D, T], F32, kind="ExternalInput").ap()
    w = nc.dram_tensor("w", [D, E], F32, kind="ExternalInput").ap()
    b = nc.dram_tensor("b", [E], F32, kind="ExternalInput").ap()
    top_idx = nc.dram_tensor("top_idx", [T, 2], U32, kind="ExternalOutput").ap()
    p_part = nc.dram_tensor("p_part", [1, E], F32, kind="ExternalOutput").ap()

    with tile.TileContext(nc) as tc:
        with ExitStack() as ctx:
            _build_kernel_body(ctx, tc, xt, w, b, top_idx, p_part)
    nc.compile()
    _COMPILED_NC = nc
    return nc


def _run_device(x, W, b, trace=False):
    nc = _get_compiled()
    xf = np.ascontiguousarray(np.asarray(x, dtype=np.float32)).reshape(B * S, D)
    Wf = np.ascontiguousarray(np.asarray(W, dtype=np.float32))
    bf = np.ascontiguousarray(np.asarray(b, dtype=np.float32))
    in_maps = []
    for c in range(N_CORES):
        shard = np.ascontiguousarray(xf[c * T : (c + 1) * T, :].T)
        in_maps.append({"xt": shard, "w": Wf, "b": bf})
    res = bass_utils.run_bass_kernel_spmd(
        nc, in_maps, core_ids=list(range(N_CORES)), trace=trace
    )
    return res


def kernel(x, W, b):
    res = _run_device(x, W, b, trace=False)
    return _assemble(res.results)


def _assemble(results):
    top_u = np.concatenate(
        [results[c]["top_idx"] for c in range(N_CORES)], axis=0
    )  # [B*S, 2] uint32
    top_idx = top_u.astype(np.int32).reshape(B, S, 2)

    p_sum = np.zeros(E, dtype=np.float64)
    for c in range(N_CORES):
        p_sum += results[c]["p_part"][0].astype(np.float64)
    P_i = p_sum / float(B)
    f_i = np.bincount(top_u.ravel().astype(np.int64), minlength=E).astype(
        np.float64
    ) / float(B)
    aux_loss = np.float32(0.01 * E * np.sum(f_i * P_i))

    routed_experts = np.zeros((B, S, E), dtype=np.float32)
    routed_probs = np.zeros((B, S, E), dtype=np.float32)
    return routed_experts, routed_probs, top_idx, aux_loss


# revision 11
# speedup vs baseline: 1.2471x; 1.0884x over previous
"""MoE ExpertAllocation router kernel for Trainium2 (8 NeuronCores, Bass/Tile).

Reference math (B=8, S=2048, D=4096, E=64):
  logits       = x @ W + b                         [B,S,E]
  router_probs = softmax(logits, -1)
  top_idx      = top_k(router_probs, 2).indices    [B,S,2] int32
  f_i          = one-hot-top2 counts / B
  P_i          = router_probs.sum((0,1)) / B
  aux_loss     = 0.01 * E * sum(f_i * P_i)
  capacity mask: buffer_size = (B/E)*1.25 = 0.15625 < 1, and the inclusive
  cumsum of the 0/1 routed_experts is >= 1 wherever routed_experts == 1, so
  expert_mask zeroes every routed entry:
    routed_experts (returned) == 0 and routed_probs == 0 identically.

Device work per core (tokens sharded 8 x 2048):
  - fp32 GEMM, W_k [128,64] stationary, x^T streamed; two k-chains run
    concurrently on PE column halves h0/h1 (2x col tiling), partial halves
    summed during PSUM evacuation
  - GEMM runs in two half-token phases so phase-1 softmax/top-2 stats
    overlap phase-2 GEMM DMA/compute
  - PE transpose of logits^T -> [128 tok, 64 expert] tiles
  - DVE max8/max_index -> top-2 indices (tie order matches jax top_k)
  - ACT exp(x - max) with fused row-sum; DVE reciprocal
  - P_i partial = sum_t exp[t,e] * recip[t] via tiny PE matmuls
Host: shards/transposes x, packs W into [128, KT*E], reduces P_i partials,
bincounts f_i, assembles outputs.
"""

import os
import sys

import numpy as np

for _p in ("/opt/trn_rl_repo", os.path.expanduser("~/.axon_site/_ro/trn_rl_repo")):
    if os.path.isdir(_p) and _p not in sys.path:
        sys.path.append(_p)

import concourse.bass as bass  # noqa: E402
import concourse.tile as tile  # noqa: E402
from concourse import bacc, mybir  # noqa: E402
from concourse import bass_utils  # noqa: E402
from concourse.masks import make_identity  # noqa: E402

B, S, D, E = 8, 2048, 4096, 64
N_CORES = 8
T = (B * S) // N_CORES  # tokens per core = 2048
P = 128
KT = D // P  # 32 k-tiles
NT = T // P  # 16 token tiles of 128
GF = 512  # GEMM moving free dim (fp32 max, one PSUM bank)
TB = T // GF  # 4 token blocks
PHASES = 2
TBP = TB // PHASES  # token blocks per phase
TPH = T // PHASES  # tokens per phase (1024)
NTP = NT // PHASES  # 128-token tiles per phase (8)

F32 = mybir.dt.float32
U32 = mybir.dt.uint32


def _build_kernel_body(ctx, tc, xt, w, b, top_idx, p_part):
    nc = tc.nc

    const_pool = ctx.enter_context(tc.tile_pool(name="const", bufs=1))
    xpool = ctx.enter_context(tc.tile_pool(name="xtiles", bufs=6))
    gpsum = ctx.enter_context(tc.tile_pool(name="gpsum", bufs=TB, space="PSUM"))
    tpsum = ctx.enter_context(tc.tile_pool(name="tpsum", bufs=2, space="PSUM"))
    ppsum = ctx.enter_context(tc.tile_pool(name="ppsum", bufs=1, space="PSUM"))
    work = ctx.enter_context(tc.tile_pool(name="work", bufs=1))
    small = ctx.enter_context(tc.tile_pool(name="small", bufs=4))

    # W is host-packed to [128, KT*E] (w_packed[p, k*E+e] = W[k*128+p, e]) so
    # it loads as one fully-contiguous DMA; it rides the ScalarE HWDGE ring
    # (qActDynamicHW) so the Sync ring streams x^T from the first trigger.
    w_sb = const_pool.tile([P, KT, E], F32)
    nc.scalar.dma_start(w_sb, w.rearrange("p (ko e) -> p ko e", e=E))
    b_sb = const_pool.tile([E, 1], F32)
    nc.gpsimd.dma_start(b_sb, b[:, None])
    ident = const_pool.tile([E, E], F32)
    make_identity(nc, ident)

    # Persistent work tiles
    logitsT = work.tile([E, T], F32)
    logits3 = work.tile([P, NT, E], F32)
    ex3 = work.tile([P, NT, E], F32)
    rec = work.tile([P, NT], F32)
    idxacc = work.tile([P, NT, 2], U32)

    gps = [
        gpsum.tile([P, GF], F32, tag="gps", name=f"gps{i}") for i in range(TB)
    ]
    pp = ppsum.tile([1, E], F32)
    KP = KT // 2

    def gemm_phase(ph):
        c0 = ph * TPH  # column (token) offset of this phase
        for j in range(KP):
            k0, k1 = 2 * j, 2 * j + 1
            xt_e = xpool.tile([P, TPH], F32, tag="xt", name=f"xte{ph}_{j}")
            nc.sync.dma_start(xt_e, xt[k0 * P : (k0 + 1) * P, c0 : c0 + TPH])
            xt_o = xpool.tile([P, TPH], F32, tag="xt", name=f"xto{ph}_{j}")
            nc.sync.dma_start(xt_o, xt[k1 * P : (k1 + 1) * P, c0 : c0 + TPH])
            for tb in range(TBP):
                g = gps[ph * TBP + tb]
                nc.tensor.matmul(
                    g[0:E, :],
                    lhsT=w_sb[:, k0, :],
                    rhs=xt_e[:, tb * GF : (tb + 1) * GF],
                    start=(j == 0),
                    stop=(j == KP - 1),
                    tile_position=(0, 0),
                    skip_group_check=True,
                )
                nc.tensor.matmul(
                    g[E : 2 * E, :],
                    lhsT=w_sb[:, k1, :],
                    rhs=xt_o[:, tb * GF : (tb + 1) * GF],
                    start=(j == 0),
                    stop=(j == KP - 1),
                    tile_position=(0, E),
                    skip_group_check=True,
                )

    def stats_phase(ph):
        # PSUM -> SBUF with bias add + h0/h1 combine (DVE reads one PSUM
        # operand per op)
        for tb in range(TBP):
            g = gps[ph * TBP + tb]
            seg = logitsT[:, ph * TPH + tb * GF : ph * TPH + (tb + 1) * GF]
            nc.vector.tensor_scalar(
                out=seg,
                in0=g[0:E, :],
                scalar1=b_sb,
                scalar2=None,
                op0=mybir.AluOpType.add,
            )
            nc.vector.tensor_add(out=seg, in0=seg, in1=g[E : 2 * E, :])

        for ti in range(NTP):
            t = ph * NTP + ti
            tp = tpsum.tile([P, E], F32, tag="tp")
            nc.tensor.transpose(tp, logitsT[:, t * P : (t + 1) * P], ident)
            # Alternate the PSUM evacuation copy between ScalarE and DVE to
            # balance the per-tile pipeline across engines.
            if ti % 2 == 0:
                nc.scalar.copy(out=logits3[:, t, :], in_=tp)
            else:
                nc.vector.tensor_copy(out=logits3[:, t, :], in_=tp)

        for ti in range(NTP):
            t = ph * NTP + ti
            lg = logits3[:, t, :]
            mx8 = small.tile([P, 8], F32, tag="mx8")
            nc.vector.max(out=mx8, in_=lg)
            ix8 = small.tile([P, 8], U32, tag="ix8")
            nc.vector.max_index(out=ix8, in_max=mx8, in_values=lg)
            nc.gpsimd.tensor_copy(out=idxacc[:, t, :], in_=ix8[:, 0:2])
            negmx = small.tile([P, 1], F32, tag="negmx")
            nc.gpsimd.tensor_scalar_mul(negmx, mx8[:, 0:1], -1.0)
            ssum = small.tile([P, 1], F32, tag="ssum")
            nc.scalar.activation(
                out=ex3[:, t, :],
                in_=lg,
                func=mybir.ActivationFunctionType.Exp,
                bias=negmx,
                scale=1.0,
                accum_out=ssum,
            )
            nc.vector.reciprocal(out=rec[:, t : t + 1], in_=ssum)

        # P_i partials accumulate into one PSUM row across both phases
        for ti in range(NTP):
            t = ph * NTP + ti
            nc.tensor.matmul(
                pp,
                lhsT=rec[:, t : t + 1],
                rhs=ex3[:, t, :],
                start=(t == 0),
                stop=(t == NT - 1),
                skip_group_check=True,
            )

        # Emit this phase's top-2 indices: SBUF [p, i, j] -> DRAM [(i p), j]
        nc.sync.dma_start(
            top_idx[ph * TPH : (ph + 1) * TPH, :].rearrange(
                "(i p) j -> p i j", p=P
            ),
            idxacc[:, ph * NTP : (ph + 1) * NTP, :],
        )

    for ph in range(PHASES):
        gemm_phase(ph)
        stats_phase(ph)

    p_sb = small.tile([1, E], F32, tag="pout")
    nc.vector.tensor_copy(out=p_sb, in_=pp)
    nc.sync.dma_start(p_part, p_sb)


_COMPILED_NC = None


def _get_compiled():
    global _COMPILED_NC
    if _COMPILED_NC is not None:
        return _COMPILED_NC
    from contextlib import ExitStack

    nc = bacc.Bacc(
        "TRN2",
        target_bir_lowering=False,
        debug=False,
        enable_asserts=False,
        num_devices=N_CORES,
    )
    xt = nc.dram_tensor("xt", [D, T], F32, kind="ExternalInput").ap()
    w = nc.dram_tensor("w", [P, KT * E], F32, kind="ExternalInput").ap()
    b = nc.dram_tensor("b", [E], F32, kind="ExternalInput").ap()
    top_idx = nc.dram_tensor("top_idx", [T, 2], U32, kind="ExternalOutput").ap()
    p_part = nc.dram_tensor("p_part", [1, E], F32, kind="ExternalOutput").ap()

    with tile.TileContext(nc) as tc:
        with ExitStack() as ctx:
            _build_kernel_body(ctx, tc, xt, w, b, top_idx, p_part)
    nc.compile()
    _COMPILED_NC = nc
    return nc


def _run_device(x, W, b, trace=False):
    nc = _get_compiled()
    xf = np.ascontiguousarray(np.asarray(x, dtype=np.float32)).reshape(B * S, D)
    Wf = np.asarray(W, dtype=np.float32)
    # w_packed[p, k*E+e] = W[k*128+p, e]
    w_packed = np.ascontiguousarray(
        Wf.reshape(KT, P, E).transpose(1, 0, 2).reshape(P, KT * E)
    )
    bf = np.ascontiguousarray(np.asarray(b, dtype=np.float32))
    in_maps = []
    for c in range(N_CORES):
        shard = np.ascontiguousarray(xf[c * T : (c + 1) * T, :].T)
        in_maps.append({"xt": shard, "w": w_packed, "b": bf})
    res = bass_utils.run_bass_kernel_spmd(
        nc, in_maps, core_ids=list(range(N_CORES)), trace=trace
    )
    return res


def kernel(x, W, b):
    res = _run_device(x, W, b, trace=False)
    return _assemble(res.results)


def _assemble(results):
    top_u = np.concatenate(
        [results[c]["top_idx"] for c in range(N_CORES)], axis=0
    )  # [B*S, 2] uint32
    top_idx = top_u.astype(np.int32).reshape(B, S, 2)

    p_sum = np.zeros(E, dtype=np.float64)
    for c in range(N_CORES):
        p_sum += results[c]["p_part"][0].astype(np.float64)
    P_i = p_sum / float(B)
    f_i = np.bincount(top_u.ravel().astype(np.int64), minlength=E).astype(
        np.float64
    ) / float(B)
    aux_loss = np.float32(0.01 * E * np.sum(f_i * P_i))

    routed_experts = np.zeros((B, S, E), dtype=np.float32)
    routed_probs = np.zeros((B, S, E), dtype=np.float32)
    return routed_experts, routed_probs, top_idx, aux_loss


# revision 13
# speedup vs baseline: 1.2736x; 1.0213x over previous
"""MoE ExpertAllocation router kernel for Trainium2 (8 NeuronCores, Bass/Tile).

Reference math (B=8, S=2048, D=4096, E=64):
  logits       = x @ W + b                         [B,S,E]
  router_probs = softmax(logits, -1)
  top_idx      = top_k(router_probs, 2).indices    [B,S,2] int32
  f_i          = one-hot-top2 counts / B
  P_i          = router_probs.sum((0,1)) / B
  aux_loss     = 0.01 * E * sum(f_i * P_i)
  capacity mask: buffer_size = (B/E)*1.25 = 0.15625 < 1, and the inclusive
  cumsum of the 0/1 routed_experts is >= 1 wherever routed_experts == 1, so
  expert_mask zeroes every routed entry:
    routed_experts (returned) == 0 and routed_probs == 0 identically.

Device work per core (tokens sharded 8 x 2048):
  - fp32 GEMM, W_k [128,64] stationary, x^T streamed; two k-chains run
    concurrently on PE column halves h0/h1 (2x col tiling), partial halves
    summed during PSUM evacuation
  - GEMM runs in two half-token phases so phase-1 softmax/top-2 stats
    overlap phase-2 GEMM DMA/compute
  - PE transpose of logits^T -> [128 tok, 64 expert] tiles
  - DVE max8/max_index -> top-2 indices (tie order matches jax top_k)
  - ACT exp(x - max) with fused row-sum; DVE reciprocal
  - P_i partial = sum_t exp[t,e] * recip[t] via tiny PE matmuls
Host: shards/transposes x, packs W into [128, KT*E], reduces P_i partials,
bincounts f_i, assembles outputs.
"""

import os
import sys

import numpy as np

for _p in ("/opt/trn_rl_repo", os.path.expanduser("~/.axon_site/_ro/trn_rl_repo")):
    if os.path.isdir(_p) and _p not in sys.path:
        sys.path.append(_p)

import concourse.bass as bass  # noqa: E402
import concourse.tile as tile  # noqa: E402
from concourse import bacc, mybir  # noqa: E402
from concourse import bass_utils  # noqa: E402
from concourse.masks import make_identity  # noqa: E402

B, S, D, E = 8, 2048, 4096, 64
N_CORES = 8
T = (B * S) // N_CORES  # tokens per core = 2048
P = 128
KT = D // P  # 32 k-tiles
NT = T // P  # 16 token tiles of 128
GF = 512  # GEMM moving free dim (fp32 max, one PSUM bank)
TB = T // GF  # 4 token blocks
PHASES = 2
TBP = TB // PHASES  # token blocks per phase
TPH = T // PHASES  # tokens per phase (1024)
NTP = NT // PHASES  # 128-token tiles per phase (8)

F32 = mybir.dt.float32
U32 = mybir.dt.uint32


def _build_kernel_body(ctx, tc, xt, w, b, top_idx, p_part):
    nc = tc.nc

    const_pool = ctx.enter_context(tc.tile_pool(name="const", bufs=1))
    xpool = ctx.enter_context(tc.tile_pool(name="xtiles", bufs=10))
    gpsum = ctx.enter_context(tc.tile_pool(name="gpsum", bufs=TB, space="PSUM"))
    tpsum = ctx.enter_context(tc.tile_pool(name="tpsum", bufs=2, space="PSUM"))
    ppsum = ctx.enter_context(tc.tile_pool(name="ppsum", bufs=1, space="PSUM"))
    work = ctx.enter_context(tc.tile_pool(name="work", bufs=1))
    small = ctx.enter_context(tc.tile_pool(name="small", bufs=4))

    # W is host-packed to [128, KT*E] (w_packed[p, k*E+e] = W[k*128+p, e]) so
    # it loads as one fully-contiguous DMA; it rides the ScalarE HWDGE ring
    # (qActDynamicHW) so the Sync ring streams x^T from the first trigger.
    w_sb = const_pool.tile([P, KT, E], F32)
    nc.scalar.dma_start(w_sb, w.rearrange("p (ko e) -> p ko e", e=E))
    b_sb = const_pool.tile([E, 1], F32)
    nc.gpsimd.dma_start(b_sb, b[:, None])
    ident = const_pool.tile([E, E], F32)
    make_identity(nc, ident)

    # Persistent work tiles
    logitsT = work.tile([E, T], F32)
    logits3 = work.tile([P, NT, E], F32)
    ex3 = work.tile([P, NT, E], F32)
    rec = work.tile([P, NT], F32)
    idxacc = work.tile([P, NT, 2], U32)

    gps = [
        gpsum.tile([P, GF], F32, tag="gps", name=f"gps{i}") for i in range(TB)
    ]
    pp = ppsum.tile([1, E], F32)
    KP = KT // 2

    def gemm_phase(ph):
        c0 = ph * TPH  # column (token) offset of this phase
        for j in range(KP):
            k0, k1 = 2 * j, 2 * j + 1
            # Alternate the two x^T streams across the SP and ACT HWDGE
            # rings: one ring's FIFO + per-DMA overhead caps at ~310 GB/s,
            # two rings together sustain the ~358 GB/s HBM limit.
            xt_e = xpool.tile([P, TPH], F32, tag="xt", name=f"xte{ph}_{j}")
            nc.sync.dma_start(xt_e, xt[k0 * P : (k0 + 1) * P, c0 : c0 + TPH])
            xt_o = xpool.tile([P, TPH], F32, tag="xt", name=f"xto{ph}_{j}")
            nc.scalar.dma_start(xt_o, xt[k1 * P : (k1 + 1) * P, c0 : c0 + TPH])
            for tb in range(TBP):
                g = gps[ph * TBP + tb]
                nc.tensor.matmul(
                    g[0:E, :],
                    lhsT=w_sb[:, k0, :],
                    rhs=xt_e[:, tb * GF : (tb + 1) * GF],
                    start=(j == 0),
                    stop=(j == KP - 1),
                    tile_position=(0, 0),
                    skip_group_check=True,
                )
                nc.tensor.matmul(
                    g[E : 2 * E, :],
                    lhsT=w_sb[:, k1, :],
                    rhs=xt_o[:, tb * GF : (tb + 1) * GF],
                    start=(j == 0),
                    stop=(j == KP - 1),
                    tile_position=(0, E),
                    skip_group_check=True,
                )

    def stats_phase(ph):
        # PSUM -> SBUF with bias add + h0/h1 combine (DVE reads one PSUM
        # operand per op)
        for tb in range(TBP):
            g = gps[ph * TBP + tb]
            seg = logitsT[:, ph * TPH + tb * GF : ph * TPH + (tb + 1) * GF]
            nc.vector.tensor_scalar(
                out=seg,
                in0=g[0:E, :],
                scalar1=b_sb,
                scalar2=None,
                op0=mybir.AluOpType.add,
            )
            nc.vector.tensor_add(out=seg, in0=seg, in1=g[E : 2 * E, :])

        for ti in range(NTP):
            t = ph * NTP + ti
            tp = tpsum.tile([P, E], F32, tag="tp")
            nc.tensor.transpose(tp, logitsT[:, t * P : (t + 1) * P], ident)
            # Alternate the PSUM evacuation copy between ScalarE and DVE to
            # balance the per-tile pipeline across engines.
            if ti % 2 == 0:
                nc.scalar.copy(out=logits3[:, t, :], in_=tp)
            else:
                nc.vector.tensor_copy(out=logits3[:, t, :], in_=tp)

        for ti in range(NTP):
            t = ph * NTP + ti
            lg = logits3[:, t, :]
            mx8 = small.tile([P, 8], F32, tag="mx8")
            nc.vector.max(out=mx8, in_=lg)
            ix8 = small.tile([P, 8], U32, tag="ix8")
            nc.vector.max_index(out=ix8, in_max=mx8, in_values=lg)
            nc.gpsimd.tensor_copy(out=idxacc[:, t, :], in_=ix8[:, 0:2])
            negmx = small.tile([P, 1], F32, tag="negmx")
            nc.gpsimd.tensor_scalar_mul(negmx, mx8[:, 0:1], -1.0)
            ssum = small.tile([P, 1], F32, tag="ssum")
            nc.scalar.activation(
                out=ex3[:, t, :],
                in_=lg,
                func=mybir.ActivationFunctionType.Exp,
                bias=negmx,
                scale=1.0,
                accum_out=ssum,
            )
            nc.vector.reciprocal(out=rec[:, t : t + 1], in_=ssum)

        # P_i partials accumulate into one PSUM row across both phases
        for ti in range(NTP):
            t = ph * NTP + ti
            nc.tensor.matmul(
                pp,
                lhsT=rec[:, t : t + 1],
                rhs=ex3[:, t, :],
                start=(t == 0),
                stop=(t == NT - 1),
                skip_group_check=True,
            )

        # Emit this phase's top-2 indices: SBUF [p, i, j] -> DRAM [(i p), j]
        nc.sync.dma_start(
            top_idx[ph * TPH : (ph + 1) * TPH, :].rearrange(
                "(i p) j -> p i j", p=P
            ),
            idxacc[:, ph * NTP : (ph + 1) * NTP, :],
        )

    for ph in range(PHASES):
        gemm_phase(ph)
        stats_phase(ph)

    p_sb = small.tile([1, E], F32, tag="pout")
    nc.vector.tensor_copy(out=p_sb, in_=pp)
    nc.sync.dma_start(p_part, p_sb)


_COMPILED_NC = None


def _get_compiled():
    global _COMPILED_NC
    if _COMPILED_NC is not None:
        return _COMPILED_NC
    from contextlib import ExitStack

    nc = bacc.Bacc(
        "TRN2",
        target_bir_lowering=False,
        debug=False,
        enable_asserts=False,
        num_devices=N_CORES,
    )
    xt = nc.dram_tensor("xt", [D, T], F32, kind="ExternalInput").ap()
    w = nc.dram_tensor("w", [P, KT * E], F32, kind="ExternalInput").ap()
    b = nc.dram_tensor("b", [E], F32, kind="ExternalInput").ap()
    top_idx = nc.dram_tensor("top_idx", [T, 2], U32, kind="ExternalOutput").ap()
    p_part = nc.dram_tensor("p_part", [1, E], F32, kind="ExternalOutput").ap()

    with tile.TileContext(nc) as tc:
        with ExitStack() as ctx:
            _build_kernel_body(ctx, tc, xt, w, b, top_idx, p_part)
    nc.compile()
    _COMPILED_NC = nc
    return nc


def _run_device(x, W, b, trace=False):
    nc = _get_compiled()
    xf = np.ascontiguousarray(np.asarray(x, dtype=np.float32)).reshape(B * S, D)
    Wf = np.asarray(W, dtype=np.float32)
    # w_packed[p, k*E+e] = W[k*128+p, e]
    w_packed = np.ascontiguousarray(
        Wf.reshape(KT, P, E).transpose(1, 0, 2).reshape(P, KT * E)
    )
    bf = np.ascontiguousarray(np.asarray(b, dtype=np.float32))
    in_maps = []
    for c in range(N_CORES):
        shard = np.ascontiguousarray(xf[c * T : (c + 1) * T, :].T)
        in_maps.append({"xt": shard, "w": w_packed, "b": bf})
    res = bass_utils.run_bass_kernel_spmd(
        nc, in_maps, core_ids=list(range(N_CORES)), trace=trace
    )
    return res


def kernel(x, W, b):
    res = _run_device(x, W, b, trace=False)
    return _assemble(res.results)


def _assemble(results):
    top_u = np.concatenate(
        [results[c]["top_idx"] for c in range(N_CORES)], axis=0
    )  # [B*S, 2] uint32
    top_idx = top_u.astype(np.int32).reshape(B, S, 2)

    p_sum = np.zeros(E, dtype=np.float64)
    for c in range(N_CORES):
        p_sum += results[c]["p_part"][0].astype(np.float64)
    P_i = p_sum / float(B)
    f_i = np.bincount(top_u.ravel().astype(np.int64), minlength=E).astype(
        np.float64
    ) / float(B)
    aux_loss = np.float32(0.01 * E * np.sum(f_i * P_i))

    routed_experts = np.zeros((B, S, E), dtype=np.float32)
    routed_probs = np.zeros((B, S, E), dtype=np.float32)
    return routed_experts, routed_probs, top_idx, aux_loss


# revision 15
# speedup vs baseline: 1.4110x; 1.1078x over previous
"""MoE ExpertAllocation router kernel for Trainium2 (8 NeuronCores, Bass/Tile).

Reference math (B=8, S=2048, D=4096, E=64):
  logits       = x @ W + b                         [B,S,E]
  router_probs = softmax(logits, -1)
  top_idx      = top_k(router_probs, 2).indices    [B,S,2] int32
  f_i          = one-hot-top2 counts / B
  P_i          = router_probs.sum((0,1)) / B
  aux_loss     = 0.01 * E * sum(f_i * P_i)
  capacity mask: buffer_size = (B/E)*1.25 = 0.15625 < 1, and the inclusive
  cumsum of the 0/1 routed_experts is >= 1 wherever routed_experts == 1, so
  expert_mask zeroes every routed entry:
    routed_experts (returned) == 0 and routed_probs == 0 identically.

Device work per core (tokens sharded 8 x 2048):
  - fp32 GEMM, W_k [128,64] stationary, x^T streamed; two k-chains run
    concurrently on PE column halves h0/h1 (2x col tiling), partial halves
    summed during PSUM evacuation
  - GEMM runs in two half-token phases so phase-1 softmax/top-2 stats
    overlap phase-2 GEMM DMA/compute
  - PE transpose of logits^T -> [128 tok, 64 expert] tiles
  - DVE max8/max_index -> top-2 indices (tie order matches jax top_k)
  - ACT exp(x - max) with fused row-sum; DVE reciprocal
  - P_i partial = sum_t exp[t,e] * recip[t] via tiny PE matmuls
Host: shards/transposes x, packs W into [128, KT*E], reduces P_i partials,
bincounts f_i, assembles outputs.
"""

import os
import sys

import numpy as np

for _p in ("/opt/trn_rl_repo", os.path.expanduser("~/.axon_site/_ro/trn_rl_repo")):
    if os.path.isdir(_p) and _p not in sys.path:
        sys.path.append(_p)

import concourse.bass as bass  # noqa: E402
import concourse.tile as tile  # noqa: E402
from concourse import bacc, mybir  # noqa: E402
from concourse import bass_utils  # noqa: E402
from concourse.masks import make_identity  # noqa: E402

B, S, D, E = 8, 2048, 4096, 64
N_CORES = 8
T = (B * S) // N_CORES  # tokens per core = 2048
P = 128
KT = D // P  # 32 k-tiles
NT = T // P  # 16 token tiles of 128
GF = 512  # GEMM moving free dim (fp32 max, one PSUM bank)
TB = T // GF  # 4 token blocks
PHASES = 2
TBP = TB // PHASES  # token blocks per phase
TPH = T // PHASES  # tokens per phase (1024)
NTP = NT // PHASES  # 128-token tiles per phase (8)

F32 = mybir.dt.float32
U32 = mybir.dt.uint32


def _build_kernel_body(ctx, tc, xt, w, b, top_idx, p_part):
    nc = tc.nc

    const_pool = ctx.enter_context(tc.tile_pool(name="const", bufs=1))
    xpool = ctx.enter_context(tc.tile_pool(name="xtiles", bufs=10))
    gpsum = ctx.enter_context(tc.tile_pool(name="gpsum", bufs=TB, space="PSUM"))
    tpsum = ctx.enter_context(tc.tile_pool(name="tpsum", bufs=2, space="PSUM"))
    ppsum = ctx.enter_context(tc.tile_pool(name="ppsum", bufs=1, space="PSUM"))
    work = ctx.enter_context(tc.tile_pool(name="work", bufs=1))
    small = ctx.enter_context(tc.tile_pool(name="small", bufs=4))

    # W is host-packed to [128, KT*E] (w_packed[p, k*E+e] = W[k*128+p, e]) so
    # it loads as fully-contiguous DMAs; it rides the ScalarE HWDGE ring
    # (qActDynamicHW) so the Sync ring streams x^T from the first trigger.
    # Two halves so the first matmuls only wait on the low-k half.
    KH = KT // 2
    w_lo = const_pool.tile([P, KH, E], F32)
    nc.scalar.dma_start(
        w_lo, w[:, : KH * E].rearrange("p (ko e) -> p ko e", e=E)
    )
    w_hi = const_pool.tile([P, KH, E], F32)
    nc.scalar.dma_start(
        w_hi, w[:, KH * E :].rearrange("p (ko e) -> p ko e", e=E)
    )

    def wk(k):
        return w_lo[:, k, :] if k < KH else w_hi[:, k - KH, :]

    b_sb = const_pool.tile([E, 1], F32)
    nc.gpsimd.dma_start(b_sb, b[:, None])
    ident = const_pool.tile([E, E], F32)
    make_identity(nc, ident)

    # PE warmup: ~5us of dummy matmuls so HAM reaches K=8/8 before the real
    # GEMM starts (cold first matmuls at 1.2 GHz ripple backpressure into
    # the DMA pipeline).  ident is the only dependency; results are never
    # read.
    wm_ps = ppsum.tile([E, E], F32, name="warm_ps")
    for _ in range(24):
        nc.tensor.matmul(wm_ps, lhsT=ident, rhs=ident, start=True, stop=True)

    # Persistent work tiles
    logitsT = work.tile([E, T], F32)
    logits3 = work.tile([P, NT, E], F32)
    ex3 = work.tile([P, NT, E], F32)
    rec = work.tile([P, NT], F32)
    idxacc = work.tile([P, NT, 2], U32)

    gps = [
        gpsum.tile([P, GF], F32, tag="gps", name=f"gps{i}") for i in range(TB)
    ]
    pp = ppsum.tile([1, E], F32)
    KP = KT // 2

    def gemm_phase(ph):
        c0 = ph * TPH  # column (token) offset of this phase
        for j in range(KP):
            k0, k1 = 2 * j, 2 * j + 1
            # Alternate the two x^T streams across the SP and ACT HWDGE
            # rings: one ring's FIFO + per-DMA overhead caps at ~310 GB/s,
            # two rings together sustain the ~358 GB/s HBM limit.
            xt_e = xpool.tile([P, TPH], F32, tag="xt", name=f"xte{ph}_{j}")
            nc.sync.dma_start(xt_e, xt[k0 * P : (k0 + 1) * P, c0 : c0 + TPH])
            xt_o = xpool.tile([P, TPH], F32, tag="xt", name=f"xto{ph}_{j}")
            nc.scalar.dma_start(xt_o, xt[k1 * P : (k1 + 1) * P, c0 : c0 + TPH])
            for tb in range(TBP):
                g = gps[ph * TBP + tb]
                nc.tensor.matmul(
                    g[0:E, :],
                    lhsT=wk(k0),
                    rhs=xt_e[:, tb * GF : (tb + 1) * GF],
                    start=(j == 0),
                    stop=(j == KP - 1),
                    tile_position=(0, 0),
                    skip_group_check=True,
                )
                nc.tensor.matmul(
                    g[E : 2 * E, :],
                    lhsT=wk(k1),
                    rhs=xt_o[:, tb * GF : (tb + 1) * GF],
                    start=(j == 0),
                    stop=(j == KP - 1),
                    tile_position=(0, E),
                    skip_group_check=True,
                )

    def stats_phase(ph):
        # PSUM -> SBUF with bias add + h0/h1 combine (DVE reads one PSUM
        # operand per op)
        for tb in range(TBP):
            g = gps[ph * TBP + tb]
            seg = logitsT[:, ph * TPH + tb * GF : ph * TPH + (tb + 1) * GF]
            nc.vector.tensor_scalar(
                out=seg,
                in0=g[0:E, :],
                scalar1=b_sb,
                scalar2=None,
                op0=mybir.AluOpType.add,
            )
            nc.vector.tensor_add(out=seg, in0=seg, in1=g[E : 2 * E, :])

        for ti in range(NTP):
            t = ph * NTP + ti
            tp = tpsum.tile([P, E], F32, tag="tp")
            nc.tensor.transpose(tp, logitsT[:, t * P : (t + 1) * P], ident)
            # Alternate the PSUM evacuation copy between ScalarE and DVE to
            # balance the per-tile pipeline across engines.
            if ti % 2 == 0:
                nc.scalar.copy(out=logits3[:, t, :], in_=tp)
            else:
                nc.vector.tensor_copy(out=logits3[:, t, :], in_=tp)

        for ti in range(NTP):
            t = ph * NTP + ti
            lg = logits3[:, t, :]
            mx8 = small.tile([P, 8], F32, tag="mx8")
            nc.vector.max(out=mx8, in_=lg)
            ix8 = small.tile([P, 8], U32, tag="ix8")
            nc.vector.max_index(out=ix8, in_max=mx8, in_values=lg)
            nc.gpsimd.tensor_copy(out=idxacc[:, t, :], in_=ix8[:, 0:2])
            negmx = small.tile([P, 1], F32, tag="negmx")
            nc.gpsimd.tensor_scalar_mul(negmx, mx8[:, 0:1], -1.0)
            ssum = small.tile([P, 1], F32, tag="ssum")
            nc.scalar.activation(
                out=ex3[:, t, :],
                in_=lg,
                func=mybir.ActivationFunctionType.Exp,
                bias=negmx,
                scale=1.0,
                accum_out=ssum,
            )
            nc.vector.reciprocal(out=rec[:, t : t + 1], in_=ssum)

        # P_i partials accumulate into one PSUM row across both phases
        for ti in range(NTP):
            t = ph * NTP + ti
            nc.tensor.matmul(
                pp,
                lhsT=rec[:, t : t + 1],
                rhs=ex3[:, t, :],
                start=(t == 0),
                stop=(t == NT - 1),
                skip_group_check=True,
            )

        # Emit this phase's top-2 indices: SBUF [p, i, j] -> DRAM [(i p), j]
        nc.sync.dma_start(
            top_idx[ph * TPH : (ph + 1) * TPH, :].rearrange(
                "(i p) j -> p i j", p=P
            ),
            idxacc[:, ph * NTP : (ph + 1) * NTP, :],
        )

    for ph in range(PHASES):
        gemm_phase(ph)
        stats_phase(ph)

    p_sb = small.tile([1, E], F32, tag="pout")
    nc.vector.tensor_copy(out=p_sb, in_=pp)
    nc.sync.dma_start(p_part, p_sb)


_COMPILED_NC = None


def _get_compiled():
    global _COMPILED_NC
    if _COMPILED_NC is not None:
        return _COMPILED_NC
    from contextlib import ExitStack

    nc = bacc.Bacc(
        "TRN2",
        target_bir_lowering=False,
        debug=False,
        enable_asserts=False,
        num_devices=N_CORES,
    )
    xt = nc.dram_tensor("xt", [D, T], F32, kind="ExternalInput").ap()
    w = nc.dram_tensor("w", [P, KT * E], F32, kind="ExternalInput").ap()
    b = nc.dram_tensor("b", [E], F32, kind="ExternalInput").ap()
    top_idx = nc.dram_tensor("top_idx", [T, 2], U32, kind="ExternalOutput").ap()
    p_part = nc.dram_tensor("p_part", [1, E], F32, kind="ExternalOutput").ap()

    with tile.TileContext(nc) as tc:
        with ExitStack() as ctx:
            _build_kernel_body(ctx, tc, xt, w, b, top_idx, p_part)
    nc.compile()
    _COMPILED_NC = nc
    return nc


def _run_device(x, W, b, trace=False):
    nc = _get_compiled()
    xf = np.ascontiguousarray(np.asarray(x, dtype=np.float32)).reshape(B * S, D)
    Wf = np.asarray(W, dtype=np.float32)
    # w_packed[p, k*E+e] = W[k*128+p, e]
    w_packed = np.ascontiguousarray(
        Wf.reshape(KT, P, E).transpose(1, 0, 2).reshape(P, KT * E)
    )
    bf = np.ascontiguousarray(np.asarray(b, dtype=np.float32))
    in_maps = []
    for c in range(N_CORES):
        shard = np.ascontiguousarray(xf[c * T : (c + 1) * T, :].T)
        in_maps.append({"xt": shard, "w": w_packed, "b": bf})
    res = bass_utils.run_bass_kernel_spmd(
        nc, in_maps, core_ids=list(range(N_CORES)), trace=trace
    )
    return res


def kernel(x, W, b):
    res = _run_device(x, W, b, trace=False)
    return _assemble(res.results)


def _assemble(results):
    top_u = np.concatenate(
        [results[c]["top_idx"] for c in range(N_CORES)], axis=0
    )  # [B*S, 2] uint32
    top_idx = top_u.astype(np.int32).reshape(B, S, 2)

    p_sum = np.zeros(E, dtype=np.float64)
    for c in range(N_CORES):
        p_sum += results[c]["p_part"][0].astype(np.float64)
    P_i = p_sum / float(B)
    f_i = np.bincount(top_u.ravel().astype(np.int64), minlength=E).astype(
        np.float64
    ) / float(B)
    aux_loss = np.float32(0.01 * E * np.sum(f_i * P_i))

    routed_experts = np.zeros((B, S, E), dtype=np.float32)
    routed_probs = np.zeros((B, S, E), dtype=np.float32)
    return routed_experts, routed_probs, top_idx, aux_loss
